# revision 2
# baseline (speedup 1.0000x reference)
"""Trainium2 Bass kernel for nn_MultiHeadedSelfAttention_86388972192276 (v3).

Sharding: 8 cores = 2 batches x 4 head-groups (4 heads each). Fully data
parallel, no collectives.

v3 over v2.1 (175us):
  - Global software pipeline: scores are emitted LEAD=2 kt-units ahead
    across step boundaries, so ACT/DVE exp never drain at a (pr,c)
    boundary and the PE interleaves next-step scores with this step's
    numerator tail.
  - exp split 8 ACT / 8 DVE per step (alternating), all projection
    PSUM->SBUF copies on ACT, hps evacuation hh0->ACT hh1->DVE at the
    end of the producing step.
  - No global outT prefill: per-chunk pqm0->outT DRAM prefill during
    the previous step + accum-DMA of the attention term; final step
    uses an SBUF add + plain write (no trailing accum RMW).
  - Tail uses the row-form reciprocal path (no Ln/Exp ACT table
    reloads).
  - Startup: first kproj needs only wk half 0 + pv[:, :, 0:512]; DMA
    order puts those first.

Bias handling (exact, host-folded):
  bk: shifts every score of a query equally -> softmax-invariant, drop.
  bq: adds bq.k'_j/8 per KEY -> folded into logm_eff / b8_eff.
  bv: h_full/l = h/l + bv -> folded into pqm0 as + bv*w.

Scale folding: wq_eff=16Wq, wk_eff=16Wk -> scores PSUM = 2048*s_true;
  wv_eff=32Wv and ones-col=32 -> v_dev = 32v, cancels in h/l.
"""

import sys
import numpy as np

sys.path.insert(0, "/opt/trn_rl_repo")

B, SQ, SV = 2, 2048, 2048
DV, DQ, DK, DO, H = 1024, 1280, 1024, 1024, 16
DH = 64
NCORES = 8
HPC = 4
NEG_MASK = -30000.0

SQW = 16.0
SKW = 16.0
SVW = 32.0
SSC = SQW * SKW * 8.0        # PSUM score = SSC * s_true
A8 = 8.0 / np.log(2.0)       # 11.5416...
B8 = 7.0 * 8.0 - 0.344       # e4m3 exp bias<<3, schraudolph-tuned

# engine per key-tile for exp: 'A' = ACT, 'D' = DVE (8/8 alternating)
ENG = ['A', 'D'] * 8

_CACHE = {}


def _build_nc():
    import concourse.bass as bass
    import concourse.tile as tile
    import concourse.mybir as mybir
    from concourse import bacc
    from contextlib import ExitStack

    fp32 = mybir.dt.float32
    fp8 = mybir.dt.float8e4
    i8 = mybir.dt.int8
    AF = mybir.ActivationFunctionType
    ALU = mybir.AluOpType
    DR = mybir.MatmulPerfMode.DoubleRow

    nc = bacc.Bacc(None)

    pqT = nc.dram_tensor("pqT", [128, 10, SQ], fp8, kind="ExternalInput")
    pvkT = nc.dram_tensor("pvkT", [128, 8, SV], fp8, kind="ExternalInput")
    wq_d = nc.dram_tensor("wq", [128, 10, 256], fp8, kind="ExternalInput")
    wk_d = nc.dram_tensor("wk", [128, 8, 256], fp8, kind="ExternalInput")
    wv_d = nc.dram_tensor("wv", [128, 8, 256], fp8, kind="ExternalInput")
    logm_d = nc.dram_tensor("logm", [128, 16], fp32, kind="ExternalInput")
    b8_d = nc.dram_tensor("b8", [128, 16], fp32, kind="ExternalInput")
    wg_d = nc.dram_tensor("wg", [128, HPC, 4, 4], fp32, kind="ExternalInput")
    pqm0_d = nc.dram_tensor("pqm0", [HPC * DH, SQ], fp32, kind="ExternalInput")
    outT = nc.dram_tensor("outT", [HPC * DH, SQ], fp32, kind="ExternalOutput")

    with tile.TileContext(nc) as tc, ExitStack() as ctx:
        const = ctx.enter_context(tc.tile_pool(name="const", bufs=1))
        persist = ctx.enter_context(tc.tile_pool(name="persist", bufs=1))
        wpool = ctx.enter_context(tc.tile_pool(name="wpool", bufs=1))
        stream = ctx.enter_context(tc.tile_pool(name="stream", bufs=2))
        qstream = ctx.enter_context(tc.tile_pool(name="qstream", bufs=2))
        epool = ctx.enter_context(tc.tile_pool(name="epool", bufs=4))
        rows = ctx.enter_context(tc.tile_pool(name="rows", bufs=2))
        bcast = ctx.enter_context(tc.tile_pool(name="bcast", bufs=2))
        blend = ctx.enter_context(tc.tile_pool(name="blend", bufs=2))
        dscr = ctx.enter_context(tc.tile_pool(name="dscr", bufs=4,
                                              space="DRAM"))
        # PSUM (8 banks): scores 3x2 + numerator 2x1
        scps = ctx.enter_context(tc.tile_pool(name="scps", bufs=3,
                                              space="PSUM"))
        hps_p = ctx.enter_context(tc.tile_pool(name="hps", bufs=1,
                                               space="PSUM"))

        # ---- persistent activations (fp8) ----
        qT2 = [persist.tile([128, SQ], fp8, tag=f"qT2_{p}", name=f"qT2_{p}")
               for p in range(2)]
        kT2 = [persist.tile([128, SV], fp8, tag=f"kT2_{p}", name=f"kT2_{p}")
               for p in range(2)]
        # v_all[kk, svt, ch, 68]; col 64 = 32-ones (memset once);
        # cols 65-67 pad the ch-block stride to 272 B (DR LDW needs
        # k-tile step % 16 == 0)
        v_all = persist.tile([128, 16, HPC, 68], fp8, tag="v_all")
        nc.gpsimd.memset(v_all[:, :, :, 64], float(SVW))

        # ---- weights + inputs; first-kproj deps (wk half 0, pv cols
        # 0:512) land first ----
        wk_sb = wpool.tile([128, 8, 256], fp8)
        nc.sync.dma_start(wk_sb[:, 0:4, :], wk_d[:, 0:4, :])
        pv_cs = []
        for cv in range(2):
            pv_c = stream.tile([128, 8, 1024], fp8, tag="pv", name=f"pv{cv}")
            for j in range(2):
                eng = nc.scalar if cv == 0 else nc.gpsimd
                eng.dma_start(
                    pv_c[:, :, bass.ds(j * 512, 512)],
                    pvkT[:, :, bass.ds(cv * 1024 + j * 512, 512)])
            pv_cs.append(pv_c)
        nc.sync.dma_start(wk_sb[:, 4:8, :], wk_d[:, 4:8, :])
        wq_sb = wpool.tile([128, 10, 256], fp8)
        nc.sync.dma_start(wq_sb[:], wq_d[:])
        wv_sb = wpool.tile([128, 8, 256], fp8)
        nc.sync.dma_start(wv_sb[:], wv_d[:])
        logm_sb = const.tile([128, 16], fp32)
        nc.sync.dma_start(logm_sb[:], logm_d[:])
        b8_sb = const.tile([128, 16], fp32)
        nc.sync.dma_start(b8_sb[:], b8_d[:])
        wg_sb = const.tile([128, HPC, 4, 4], fp32)
        nc.sync.dma_start(wg_sb[:], wg_d[:])

        # ---- projections (borrow "sc" PSUM slots) ----
        def kproj_chunk(cv, j):
            for pr in range(2):
                ps = scps.tile([128, 2, 512], fp32, tag="sc")
                for t in range(4):
                    nc.tensor.matmul(
                        ps[:, 0, :],
                        wk_sb[:, 2 * t : 2 * t + 2,
                              pr * 128 : pr * 128 + 128],
                        pv_cs[cv][:, 2 * t : 2 * t + 2,
                                  bass.ds(j * 512, 512)],
                        start=(t == 0), stop=(t == 3),
                        perf_mode=DR,
                    )
                nc.scalar.copy(
                    kT2[pr][:, bass.ds(cv * 1024 + j * 512, 512)],
                    ps[:, 0, :])

        pq_cs = {}

        def load_pq_chunk(c):
            pq_c = qstream.tile([128, 10, 512], fp8, tag="pq", name=f"pq{c}")
            nc.sync.dma_start(pq_c[:], pqT[:, :, bass.ds(c * 512, 512)])
            pq_cs[c] = pq_c

        def qproj_chunk(c):
            for pr in range(2):
                ps = scps.tile([128, 2, 512], fp32, tag="sc")
                for t in range(5):
                    nc.tensor.matmul(
                        ps[:, 0, :],
                        wq_sb[:, 2 * t : 2 * t + 2,
                              pr * 128 : pr * 128 + 128],
                        pq_cs[c][:, 2 * t : 2 * t + 2, :],
                        start=(t == 0), stop=(t == 4),
                        perf_mode=DR,
                    )
                nc.scalar.copy(qT2[pr][:, bass.ds(c * 512, 512)],
                               ps[:, 0, :])
            del pq_cs[c]

        def vproj_tile(svt):
            cv, sv = divmod(svt, 8)
            ps = scps.tile([128, 2, 512], fp32, tag="sc")
            for t in range(4):
                nc.tensor.matmul(
                    ps[:, 0, 0:256],
                    pv_cs[cv][:, 2 * t : 2 * t + 2,
                              bass.ds(sv * 128, 128)],
                    wv_sb[:, 2 * t : 2 * t + 2, :],
                    start=(t == 0), stop=(t == 3),
                    perf_mode=DR,
                )
            nc.scalar.copy(
                v_all[:, svt, :, 0:64],
                ps[:, 0, 0:256].rearrange("p (c f) -> p c f", c=4))

        kproj_chunk(0, 0)
        load_pq_chunk(0)
        kproj_chunk(0, 1)
        kproj_chunk(1, 0)
        kproj_chunk(1, 1)
        qproj_chunk(0)

        # ---- attention: global pipelined loop over units g=(si,kt) ----
        combos = [(pr, c) for c in range(4) for pr in range(2)]
        LEAD = 2
        NSTEP = 8
        NG = NSTEP * 16

        ps_store = {}

        def emit_scores(g):
            si, kt = divmod(g, 16)
            pr, c = combos[si]
            ps = scps.tile([128, 2, 512], fp32, tag="sc")
            for hh in range(2):
                ro = 64 * hh
                nc.tensor.matmul(
                    ps[:, hh, :],
                    kT2[pr][ro : ro + 64, bass.ds(kt * 128, 128)],
                    qT2[pr][ro : ro + 64, bass.ds(c * 512, 512)],
                    start=True, stop=True,
                )
            ps_store[g] = ps

        def emit_exp(g, e_t):
            si, kt = divmod(g, 16)
            ps = ps_store.pop(g)
            tpl = kt & 1
            if ENG[kt] == 'A':
                nc.scalar.activation(
                    e_t[:, tpl, :, :], ps[:], AF.Exp,
                    bias=logm_sb[:, kt : kt + 1], scale=float(1.0 / SSC))
            else:
                nc.vector.tensor_scalar(
                    e_t[:, tpl, :, :].bitcast(i8), ps[:],
                    float(A8 / SSC), b8_sb[:, kt : kt + 1],
                    ALU.mult, ALU.add)

        def emit_numer(e_t, hps2, pr, dkt):
            for hh in range(2):
                nc.tensor.matmul(
                    hps2[hh][:],
                    v_all[:, 2 * dkt : 2 * dkt + 2, 2 * pr + hh, 0:65],
                    e_t[:, :, hh, :],
                    start=(dkt == 0), stop=(dkt == 7),
                    perf_mode=DR,
                )

        # ---- blend stages for step data (si, pr, c) ----
        def blend_s0(st, hps2):
            # evacuate hps PSUM (hh0 on ACT, hh1 on DVE) + l-row bounce
            st["hcpP"] = blend.tile([65, 2, 512], fp32, tag="hcpP",
                                    name="hcpP")
            nc.scalar.copy(st["hcpP"][:, 0, :], hps2[0][:])
            nc.vector.tensor_copy(st["hcpP"][:, 1, :], hps2[1][:])
            for hh in range(2):
                dmae = nc.sync if hh == 0 else nc.gpsimd
                ld = dscr.tile([1, 512], fp32, tag=f"ld{hh}", name=f"ld{hh}")
                dmae.dma_start(ld[:], st["hcpP"][64:65, hh, :])
                lz = rows.tile([128, 4], fp32, tag=f"lz{hh}", name=f"lz{hh}")
                dmae.dma_start(lz[:], ld.rearrange("o (p f) -> p (o f)", f=4))
                st[hh]["lz"] = lz

        def blend_s1(st, hh, dmae=None):
            # 1/l, *w (DVE, tiny), bounce out + broadcast into m1bP half
            s = st[hh]
            dmae = dmae or nc.sync
            if "m1bP" not in st:
                st["m1bP"] = bcast.tile([64, 2, 512], fp32, tag="m1bP",
                                        name="m1bP")
            rl = rows.tile([128, 4], fp32, tag=f"rl{hh}", name=f"rl{hh}")
            nc.vector.reciprocal(rl[:], s["lz"][:])
            m8 = rows.tile([128, 4], fp32, tag=f"m8{hh}", name=f"m8{hh}")
            nc.vector.tensor_tensor(
                m8[:], wg_sb[:, s["ch"], s["c"], :], rl[:], ALU.mult)
            md = dscr.tile([1, 512], fp32, tag=f"md{hh}", name=f"md{hh}")
            dmae.dma_start(md.rearrange("o (p f) -> p (o f)", f=4), m8[:])
            dmae.dma_start(st["m1bP"][:, hh, :],
                           md[0:1, :].to_broadcast((64, 512)))

        def blend_s2(st, pr, c, final=False):
            # a = h*(w/l) for both heads in one op; out += a (accum DMA
            # onto the per-chunk pqm0 prefill). Final step: SBUF add +
            # plain write so the kernel doesn't end on an accum RMW.
            aP = blend.tile([64, 2, 512], fp32, tag="aP", name="aP")
            eng = nc.vector if final else nc.gpsimd
            eng.tensor_tensor(
                aP[:], st["hcpP"][0:64, :, :], st["m1bP"][:], ALU.mult)
            dst = outT[bass.ds(2 * pr * 64, 128), bass.ds(c * 512, 512)]
            if final:
                oP = blend.tile([64, 2, 512], fp32, tag="oP", name="oP")
                nc.vector.tensor_tensor(oP[:], aP[:], st["btP"][:], ALU.add)
                nc.sync.dma_start(
                    dst.rearrange("(hh d) q -> d hh q", hh=2), oP[:])
            else:
                nc.gpsimd.dma_start(
                    dst.rearrange("(hh d) q -> d hh q", hh=2), aP[:],
                    accum_op=ALU.add)

        def prefill_out(pr, c):
            src = pqm0_d[bass.ds(2 * pr * 64, 128), bass.ds(c * 512, 512)]
            dst = outT[bass.ds(2 * pr * 64, 128), bass.ds(c * 512, 512)]
            nc.gpsimd.dma_start(dst, src)

        # scores lookahead prologue
        for g in range(LEAD):
            emit_scores(g)

        pending = {}   # blend state of the previous step
        prev_prc = None
        hps2 = None
        e_t = None

        for g in range(NG):
            si, kt = divmod(g, 16)
            pr, c = combos[si]
            if kt == 0:
                hps2 = [hps_p.tile([65, 512], fp32, tag=f"hT{hh}",
                                   name=f"hT{hh}")
                        for hh in range(2)]
                st_new = {0: {"ch": 2 * pr, "c": c},
                          1: {"ch": 2 * pr + 1, "c": c}}
            if g + LEAD < NG:
                emit_scores(g + LEAD)
            if kt & 1 == 0:
                e_t = epool.tile([128, 2, 2, 512], fp8, tag="e",
                                 name=f"e{si}_{kt // 2}")
            emit_exp(g, e_t)
            if si == 0:
                vproj_tile(kt)
            if kt & 1 == 1:
                emit_numer(e_t, hps2, pr, kt // 2)
            # previous step's blend stages, spread through this step
            if pending:
                if kt == 2:
                    blend_s1(pending, 0)
                elif kt == 3:
                    blend_s1(pending, 1, dmae=nc.gpsimd)
                elif kt == 5 and si < 7:
                    prefill_out(pr, c)   # prefill THIS step's chunk
                elif kt == 6:
                    blend_s2(pending, *prev_prc)
            elif kt == 5:
                prefill_out(pr, c)
            # input/projection staging for later steps
            if si == 0 and kt == 2:
                load_pq_chunk(1)
            if si == 1 and kt == 4:
                qproj_chunk(1)
            if si == 1 and kt == 8:
                load_pq_chunk(2)
            if si == 3 and kt == 4:
                qproj_chunk(2)
            if si == 3 and kt == 8:
                load_pq_chunk(3)
            if si == 5 and kt == 4:
                qproj_chunk(3)
            if si == 7 and kt == 2:
                btP = blend.tile([64, 2, 512], fp32, tag="btP", name="btP")
                src_bt = pqm0_d[bass.ds(2 * pr * 64, 128),
                                bass.ds(c * 512, 512)]
                nc.sync.dma_start(
                    btP[:], src_bt.rearrange("(hh d) q -> d hh q", hh=2))
                st_bt = btP
            # end of step: evacuate hps immediately
            if kt == 15:
                st_new2 = st_new
                blend_s0(st_new2, hps2)
                pending, prev_prc = st_new2, (pr, c)

        # final step's blend (tail) via the same cheap row path
        pending["btP"] = st_bt
        blend_s1(pending, 0)
        blend_s1(pending, 1)
        blend_s2(pending, *prev_prc, final=True)

    nc.finalize()
    return nc


def _get_nc():
    if "nc" not in _CACHE:
        _CACHE["nc"] = _build_nc()
    return _CACHE["nc"]


def _prep_core_inputs(c, pre_value_key, pre_query, value_key_masks,
                      value_key_counts, Wq, bq, Wk, bk, Wv, bv,
                      overall_gain, overall_bias):
    import ml_dtypes
    f8 = ml_dtypes.float8_e4m3
    f = np.float32

    b = c // 4
    h0 = (c % 4) * HPC
    cols = slice(h0 * DH, h0 * DH + HPC * DH)

    pqT = np.ascontiguousarray(
        pre_query[b].T.reshape(10, 128, SQ).transpose(1, 0, 2))
    pvkT = np.ascontiguousarray(
        pre_value_key[b].T.reshape(8, 128, SV).transpose(1, 0, 2))

    wq = np.ascontiguousarray(
        (Wq[:, cols] * SQW).reshape(10, 128, 256).transpose(1, 0, 2))
    wk = np.ascontiguousarray(
        (Wk[:, cols] * SKW).reshape(8, 128, 256).transpose(1, 0, 2))
    wv = np.ascontiguousarray(
        (Wv[:, cols] * SVW).reshape(8, 128, 256).transpose(1, 0, 2))

    mask_b = value_key_masks[b]
    logm = np.where(mask_b == 0, np.float32(NEG_MASK), np.float32(0.0))
    # fold bq: per-key addend bq . k'_j / 8 (k' biasless)
    u = Wk[:, cols] @ bq[cols]
    kbq = (pre_value_key[b] @ u) / 8.0
    keyb = logm + kbq.astype(np.float32)
    logm_st = np.ascontiguousarray(keyb.reshape(16, 128).T.astype(f))
    b8m = B8 + A8 * keyb
    b8_st = np.ascontiguousarray(b8m.reshape(16, 128).T.astype(f))

    # ---- host gate (exact, generic) ----
    msum = np.float32(mask_b.sum())
    km256 = (mask_b @ pre_value_key[b]) @ (Wk[:, cols] / 8.0) \
        + (bk[cols] / 8.0) * msum
    gain = overall_gain.reshape(H)
    bias = overall_bias.reshape(H)
    cnt = np.float32(value_key_counts[b])
    km2 = km256.reshape(HPC, DH)
    U = np.einsum("dhk,hk->dh", Wq[:, cols].reshape(DQ, HPC, DH), km2)
    C = (bq[cols].reshape(HPC, DH) * km2).sum(1)
    pooled = pre_query[b] @ U + C
    z = pooled * (gain[h0 : h0 + HPC] / cnt) + bias[h0 : h0 + HPC]
    w = 1.0 / (1.0 + np.exp(-z.astype(np.float64)))
    w = w.astype(np.float32)  # [SQ, HPC]

    wg = np.ascontiguousarray(
        w.T.reshape(HPC, 4, 128, 4).transpose(2, 0, 1, 3))
    pq_split = pre_query[b, :, cols].reshape(SQ, HPC, DH)
    bv_h = bv[cols].reshape(HPC, DH)
    pqm0 = pq_split * (1.0 - w)[:, :, None] + bv_h[None] * w[:, :, None]
    pqm0T = np.ascontiguousarray(pqm0.reshape(SQ, 256).T)

    return {
        "pqT": pqT.astype(f8),
        "pvkT": pvkT.astype(f8),
        "wq": np.clip(wq, -240, 240).astype(f8),
        "wk": np.clip(wk, -240, 240).astype(f8),
        "wv": np.clip(wv, -240, 240).astype(f8),
        "logm": logm_st,
        "b8": b8_st,
        "wg": wg.astype(f, copy=False),
        "pqm0": pqm0T.astype(f, copy=False),
    }


def kernel(trace=False, **inputs):
    from concourse.bass_utils import run_bass_kernel_spmd

    inputs = {k: np.asarray(v, np.float32) for k, v in inputs.items()}
    nc = _get_nc()
    in_maps = [_prep_core_inputs(c, **inputs) for c in range(NCORES)]
    res = run_bass_kernel_spmd(nc, in_maps, core_ids=list(range(NCORES)),
                               trace=trace)
    _CACHE["last_result"] = res

    pre_query = inputs["pre_query"]
    out = np.empty((B, SQ, DQ), np.float32)
    out[:, :, DO:] = pre_query[:, :, DO:]
    for c in range(NCORES):
        b = c // 4
        h0 = (c % 4) * HPC
        oT = res.results[c]["outT"]
        out[b, :, h0 * DH : h0 * DH + HPC * DH] = oT.T
    return out


# revision 10
# speedup vs baseline: 1.1039x; 1.1039x over previous
"""Trainium2 Bass kernel for nn_MultiHeadedSelfAttention_86388972192276 (v4).

Sharding: 8 cores = 2 batches x 4 head-groups (4 heads each). Fully data
parallel, no collectives.

v4 over v3 (~175us):
  - Partial-K startup: attention begins after only kproj of the first
    key half (pv cv0) + qproj(0); kproj of cv1 is woven into step 0
    (kt2/kt4). First scores ~13us instead of ~31us (startup was
    DMA-bandwidth-bound at ~180 GB/s effective).
  - Numerator pair rotation (p1..p7, p0 last): kills the hps-WAR stall
    at step starts and lets step 0 consume v tiles just-in-time.
  - qproj split into single-pr chunks, one per step (si1..si6, kt5):
    only one PSUM slot borrowed at a time.
  - Blend: both-heads-in-one bounces (ld/lz/md [.., 2, ..]), w/l
    multiply applied via gpsimd CCE accum-DMA (mult) in place on the
    evacuated hcp tile, out = hcp + btP (bf16 pqm0) on gpsimd, plain
    write. No outT prefill, no accum RMW on DRAM, no Ln/Exp tail (one
    ACT table load total).
  - hps evacuation: both hh on ACT, emitted after the next step's
    first exp (no head-of-line blocking of exps).
  - exp split 8 ACT / 8 DVE alternating.

Bias handling (exact, host-folded):
  bk: shifts every score of a query equally -> softmax-invariant, drop.
  bq: adds bq.k'_j/8 per KEY -> folded into logm_eff / b8_eff.
  bv: h_full/l = h/l + bv -> folded into pqm0 as + bv*w.

Scale folding: wq_eff=16Wq, wk_eff=16Wk -> scores PSUM = 2048*s_true;
  wv_eff=32Wv and ones-col=32 -> v_dev = 32v, cancels in h/l.
"""

import sys
import numpy as np

sys.path.insert(0, "/opt/trn_rl_repo")

B, SQ, SV = 2, 2048, 2048
DV, DQ, DK, DO, H = 1024, 1280, 1024, 1024, 16
DH = 64
NCORES = 8
HPC = 4
NEG_MASK = -30000.0

SQW = 16.0
SKW = 16.0
SVW = 32.0
SSC = SQW * SKW * 8.0        # PSUM score = SSC * s_true
A8 = 8.0 / np.log(2.0)       # 11.5416...
B8 = 7.0 * 8.0 - 0.344       # e4m3 exp bias<<3, schraudolph-tuned

# engine per key-tile for exp: 'A' = ACT, 'D' = DVE (8/8 alternating)
ENG = ['A', 'D'] * 8

_CACHE = {}


def _build_nc():
    import concourse.bass as bass
    import concourse.tile as tile
    import concourse.mybir as mybir
    from concourse import bacc
    from contextlib import ExitStack

    fp32 = mybir.dt.float32
    bf16 = mybir.dt.bfloat16
    fp8 = mybir.dt.float8e4
    i8 = mybir.dt.int8
    AF = mybir.ActivationFunctionType
    ALU = mybir.AluOpType
    DR = mybir.MatmulPerfMode.DoubleRow

    nc = bacc.Bacc(None)

    pqT = nc.dram_tensor("pqT", [128, 10, SQ], fp8, kind="ExternalInput")
    pvkT = nc.dram_tensor("pvkT", [128, 8, SV], fp8, kind="ExternalInput")
    wq_d = nc.dram_tensor("wq", [128, 10, 256], fp8, kind="ExternalInput")
    wk_d = nc.dram_tensor("wk", [128, 8, 256], fp8, kind="ExternalInput")
    wv_d = nc.dram_tensor("wv", [128, 8, 256], fp8, kind="ExternalInput")
    logm_d = nc.dram_tensor("logm", [128, 16], fp32, kind="ExternalInput")
    b8_d = nc.dram_tensor("b8", [128, 16], fp32, kind="ExternalInput")
    wg_d = nc.dram_tensor("wg", [128, HPC, 4, 4], fp32, kind="ExternalInput")
    pqm0_d = nc.dram_tensor("pqm0", [HPC * DH, SQ], bf16, kind="ExternalInput")
    outT = nc.dram_tensor("outT", [HPC * DH, SQ], fp32, kind="ExternalOutput")

    with tile.TileContext(nc) as tc, ExitStack() as ctx:
        const = ctx.enter_context(tc.tile_pool(name="const", bufs=1))
        persist = ctx.enter_context(tc.tile_pool(name="persist", bufs=1))
        wpool = ctx.enter_context(tc.tile_pool(name="wpool", bufs=1))
        stream = ctx.enter_context(tc.tile_pool(name="stream", bufs=2))
        qstream = ctx.enter_context(tc.tile_pool(name="qstream", bufs=2))
        epool = ctx.enter_context(tc.tile_pool(name="epool", bufs=5))
        rows = ctx.enter_context(tc.tile_pool(name="rows", bufs=2))
        blend = ctx.enter_context(tc.tile_pool(name="blend", bufs=2))
        dscr = ctx.enter_context(tc.tile_pool(name="dscr", bufs=2,
                                              space="DRAM"))
        # PSUM (8 banks): scores 3x2 + numerator 2x1
        scps = ctx.enter_context(tc.tile_pool(name="scps", bufs=3,
                                              space="PSUM"))
        hps_p = ctx.enter_context(tc.tile_pool(name="hps", bufs=1,
                                               space="PSUM"))

        # ---- persistent activations (fp8) ----
        qT2 = [persist.tile([128, SQ], fp8, tag=f"qT2_{p}", name=f"qT2_{p}")
               for p in range(2)]
        kT2 = [persist.tile([128, SV], fp8, tag=f"kT2_{p}", name=f"kT2_{p}")
               for p in range(2)]
        # v_all[kk, svt, ch, 68]; col 64 = 32-ones (memset once);
        # cols 65-67 pad the ch-block stride to 272 B (DR LDW needs
        # k-tile step % 16 == 0)
        v_all = persist.tile([128, 16, HPC, 68], fp8, tag="v_all")
        nc.gpsimd.memset(v_all[:, :, :, 64], float(SVW))

        # ---- weights + inputs. Startup criticals first: sync queue
        # carries wk/wq/pq0, scalar queue carries pv (cv0 then cv1). ----
        wk_sb = wpool.tile([128, 8, 256], fp8)
        nc.sync.dma_start(wk_sb[:, 0:4, :], wk_d[:, 0:4, :])
        pv_cs = []
        for cv in range(2):
            pv_c = stream.tile([128, 8, 1024], fp8, tag="pv", name=f"pv{cv}")
            for j in range(2):
                nc.scalar.dma_start(
                    pv_c[:, :, bass.ds(j * 512, 512)],
                    pvkT[:, :, bass.ds(cv * 1024 + j * 512, 512)])
            pv_cs.append(pv_c)
        nc.sync.dma_start(wk_sb[:, 4:8, :], wk_d[:, 4:8, :])
        logm_sb = const.tile([128, 16], fp32)
        nc.sync.dma_start(logm_sb[:], logm_d[:])
        b8_sb = const.tile([128, 16], fp32)
        nc.sync.dma_start(b8_sb[:], b8_d[:])
        wq_sb = wpool.tile([128, 10, 256], fp8)
        nc.sync.dma_start(wq_sb[:], wq_d[:])

        # ---- projections (borrow "sc" PSUM slots) ----
        def kproj_chunk(cv, j):
            for pr in range(2):
                ps = scps.tile([128, 2, 512], fp32, tag="sc")
                for t in range(4):
                    nc.tensor.matmul(
                        ps[:, 0, :],
                        wk_sb[:, 2 * t : 2 * t + 2,
                              pr * 128 : pr * 128 + 128],
                        pv_cs[cv][:, 2 * t : 2 * t + 2,
                                  bass.ds(j * 512, 512)],
                        start=(t == 0), stop=(t == 3),
                        perf_mode=DR,
                    )
                nc.scalar.copy(
                    kT2[pr][:, bass.ds(cv * 1024 + j * 512, 512)],
                    ps[:, 0, :])

        pq_cs = {}

        def load_pq_chunk(c):
            pq_c = qstream.tile([128, 10, 512], fp8, tag="pq", name=f"pq{c}")
            nc.sync.dma_start(pq_c[:], pqT[:, :, bass.ds(c * 512, 512)])
            pq_cs[c] = pq_c

        def qproj_half(c, pr):
            ps = scps.tile([128, 2, 512], fp32, tag="sc")
            for t in range(5):
                nc.tensor.matmul(
                    ps[:, 0, :],
                    wq_sb[:, 2 * t : 2 * t + 2,
                          pr * 128 : pr * 128 + 128],
                    pq_cs[c][:, 2 * t : 2 * t + 2, :],
                    start=(t == 0), stop=(t == 4),
                    perf_mode=DR,
                )
            nc.scalar.copy(qT2[pr][:, bass.ds(c * 512, 512)],
                           ps[:, 0, :])

        def vproj_tile(svt):
            cv, sv = divmod(svt, 8)
            ps = scps.tile([128, 2, 512], fp32, tag="sc")
            for t in range(4):
                nc.tensor.matmul(
                    ps[:, 0, 0:256],
                    pv_cs[cv][:, 2 * t : 2 * t + 2,
                              bass.ds(sv * 128, 128)],
                    wv_sb[:, 2 * t : 2 * t + 2, :],
                    start=(t == 0), stop=(t == 3),
                    perf_mode=DR,
                )
            eng = nc.scalar if svt & 1 == 0 else nc.vector
            if svt & 1 == 0:
                eng.copy(
                    v_all[:, svt, :, 0:64],
                    ps[:, 0, 0:256].rearrange("p (c f) -> p c f", c=4))
            else:
                eng.tensor_copy(
                    v_all[:, svt, :, 0:64],
                    ps[:, 0, 0:256].rearrange("p (c f) -> p c f", c=4))

        # prologue: only the cv0 half of K, q chunk 0 halves.
        # wv/wg load after pq0 on the sync queue (needed later).
        kproj_chunk(0, 0)
        load_pq_chunk(0)
        wv_sb = wpool.tile([128, 8, 256], fp8)
        nc.sync.dma_start(wv_sb[:], wv_d[:])
        wg_sb = const.tile([128, HPC, 4, 4], fp32)
        nc.sync.dma_start(wg_sb[:], wg_d[:])
        kproj_chunk(0, 1)
        qproj_half(0, 0)
        qproj_half(0, 1)

        # ---- attention: global pipelined loop over units g=(si,kt) ----
        combos = [(pr, c) for c in range(4) for pr in range(2)]
        LEAD = 2
        NG = 8 * 16

        ps_store = {}

        def emit_scores(g):
            si, kt = divmod(g, 16)
            pr, c = combos[si]
            ps = scps.tile([128, 2, 512], fp32, tag="sc")
            for hh in range(2):
                ro = 64 * hh
                nc.tensor.matmul(
                    ps[:, hh, :],
                    kT2[pr][ro : ro + 64, bass.ds(kt * 128, 128)],
                    qT2[pr][ro : ro + 64, bass.ds(c * 512, 512)],
                    start=True, stop=True,
                )
            ps_store[g] = ps

        def emit_exp(g, e_t):
            si, kt = divmod(g, 16)
            ps = ps_store.pop(g)
            tpl = kt & 1
            if ENG[kt] == 'A':
                nc.scalar.activation(
                    e_t[:, tpl, :, :], ps[:], AF.Exp,
                    bias=logm_sb[:, kt : kt + 1], scale=float(1.0 / SSC))
            else:
                nc.vector.tensor_scalar(
                    e_t[:, tpl, :, :].bitcast(i8), ps[:],
                    float(A8 / SSC), b8_sb[:, kt : kt + 1],
                    ALU.mult, ALU.add)

        # numerator pair rotation: p1..p7 first, p0 last (kt15) so the
        # hps WAR on step entry is off the critical path.
        def emit_numer(e_pair, hps2, pr, p, first, last):
            for hh in range(2):
                nc.tensor.matmul(
                    hps2[hh][:],
                    v_all[:, 2 * p : 2 * p + 2, 2 * pr + hh, 0:65],
                    e_pair[:, :, hh, :],
                    start=first, stop=last,
                    perf_mode=DR,
                )

        # ---- blend stages for the previous step's (pr, c) ----
        def blend_s0(st):
            # evacuate hps (both hh on ACT) + combined l-row bounce
            hp = st["hps"]
            st["hcpP"] = blend.tile([65, 2, 512], fp32, tag="hcpP",
                                    name="hcpP")
            nc.scalar.copy(st["hcpP"][:, 0, :], hp[0][:])
            nc.scalar.copy(st["hcpP"][:, 1, :], hp[1][:])
            ld = dscr.tile([1, 2, 512], fp32, tag="ld", name="ld")
            nc.scalar.dma_start(ld[:], st["hcpP"][64:65, :, :])
            st["ld"] = ld

        def blend_s0b(st):
            lz = rows.tile([128, 2, 4], fp32, tag="lz", name="lz")
            nc.scalar.dma_start(
                lz[:],
                st["ld"].rearrange("o hh (p f) -> p o hh f", f=4))
            st["lz"] = lz

        def blend_s1(st):
            # 1/l then *w (DVE, tiny), bounce to DRAM row form
            pr, c = st["prc"]
            rl = rows.tile([128, 2, 4], fp32, tag="rl", name="rl")
            nc.vector.reciprocal(rl[:], st["lz"][:])
            m8 = rows.tile([128, 2, 4], fp32, tag="m8", name="m8")
            nc.vector.tensor_tensor(
                m8[:], wg_sb[:, 2 * pr : 2 * pr + 2, c, :], rl[:], ALU.mult)
            md = dscr.tile([1, 2, 512], fp32, tag="md", name="md")
            nc.scalar.dma_start(
                md.rearrange("o hh (p f) -> p o hh f", f=4), m8[:])
            st["md"] = md

        def blend_s2(st):
            # broadcast w/l into a [64, 2, 512] SBUF tile
            m1b = rows.tile([64, 2, 512], fp32, tag="m1b", name="m1b")
            nc.scalar.dma_start(
                m1b[:], st["md"][0:1, :, :].to_broadcast((64, 2, 512)))
            st["m1b"] = m1b

        def blend_s3(st, final=False):
            # out = h*(w/l) + pqm0 ; plain write
            pr, c = st["prc"]
            eng = nc.vector if final else nc.gpsimd
            aP = blend.tile([64, 2, 512], fp32, tag="aP", name="aP")
            eng.tensor_tensor(aP[:], st["hcpP"][0:64, :, :], st["m1b"][:],
                              ALU.mult)
            oP = blend.tile([64, 2, 512], fp32, tag="oP", name="oP")
            eng.tensor_tensor(oP[:], aP[:], st["btP"][:], ALU.add)
            dst = outT[bass.ds(2 * pr * 64, 128), bass.ds(c * 512, 512)]
            nc.sync.dma_start(
                dst.rearrange("(hh d) q -> d hh q", hh=2), oP[:])

        # scores lookahead prologue
        for g in range(LEAD):
            emit_scores(g)

        pending = None   # blend state of the previous step
        cur = None
        hps2 = None
        e_t = None
        e_hold = None

        for g in range(NG):
            si, kt = divmod(g, 16)
            pr, c = combos[si]
            if kt == 0:
                hps2 = [hps_p.tile([65, 512], fp32, tag=f"hT{hh}",
                                   name=f"hT{hh}")
                        for hh in range(2)]
                cur = {"prc": (pr, c), "hps": hps2}
            if g + LEAD < NG:
                emit_scores(g + LEAD)
            if kt & 1 == 0:
                e_t = epool.tile([128, 2, 2, 512], fp8, tag="e",
                                 name=f"e{si}_{kt // 2}")
                if kt == 0:
                    e_hold = e_t
            emit_exp(g, e_t)
            if si == 0:
                vproj_tile(kt)
            if kt & 1 == 1 and kt >= 3:
                p = kt // 2
                emit_numer(e_t, hps2, pr, p, first=(p == 1), last=False)
                if kt == 15:
                    emit_numer(e_hold, hps2, pr, 0, first=False, last=True)
            # previous step's blend stages, spread through this step
            if pending is not None:
                if kt == 0:
                    blend_s0(pending)
                elif kt == 1:
                    blend_s0b(pending)
                elif kt == 5:
                    blend_s1(pending)
                elif kt == 6:
                    blend_s2(pending)
                elif kt == 7:
                    blend_s3(pending)
            # per-step btP prefetch (used by the NEXT step's blend_s3)
            if kt == 8:
                btP = blend.tile([64, 2, 512], bf16, tag="btP", name="btP")
                src_bt = pqm0_d[bass.ds(2 * pr * 64, 128),
                                bass.ds(c * 512, 512)]
                nc.sync.dma_start(
                    btP[:], src_bt.rearrange("(hh d) q -> d hh q", hh=2))
                cur["btP"] = btP
            # deferred cv1 K-projection + staging
            if si == 0 and kt == 2:
                kproj_chunk(1, 0)
                load_pq_chunk(1)
            if si == 0 and kt == 4:
                kproj_chunk(1, 1)
            if 1 <= si <= 6 and kt == 5:
                qc, qpr = divmod(si + 1, 2)
                if qc <= 3:
                    qproj_half(qc, qpr)
            if si in (1, 3) and kt == 10:
                load_pq_chunk(si // 2 + 2)
            if kt == 15:
                pending = cur

        # final step's blend (tail)
        blend_s0(pending)
        blend_s0b(pending)
        blend_s1(pending)
        blend_s2(pending)
        blend_s3(pending, final=True)

    nc.finalize()
    return nc


def _get_nc():
    if "nc" not in _CACHE:
        _CACHE["nc"] = _build_nc()
    return _CACHE["nc"]


def _prep_core_inputs(c, pre_value_key, pre_query, value_key_masks,
                      value_key_counts, Wq, bq, Wk, bk, Wv, bv,
                      overall_gain, overall_bias):
    import ml_dtypes
    f8 = ml_dtypes.float8_e4m3
    bf = ml_dtypes.bfloat16
    f = np.float32

    b = c // 4
    h0 = (c % 4) * HPC
    cols = slice(h0 * DH, h0 * DH + HPC * DH)

    pqT = np.ascontiguousarray(
        pre_query[b].T.reshape(10, 128, SQ).transpose(1, 0, 2))
    pvkT = np.ascontiguousarray(
        pre_value_key[b].T.reshape(8, 128, SV).transpose(1, 0, 2))

    wq = np.ascontiguousarray(
        (Wq[:, cols] * SQW).reshape(10, 128, 256).transpose(1, 0, 2))
    wk = np.ascontiguousarray(
        (Wk[:, cols] * SKW).reshape(8, 128, 256).transpose(1, 0, 2))
    wv = np.ascontiguousarray(
        (Wv[:, cols] * SVW).reshape(8, 128, 256).transpose(1, 0, 2))

    mask_b = value_key_masks[b]
    logm = np.where(mask_b == 0, np.float32(NEG_MASK), np.float32(0.0))
    # fold bq: per-key addend bq . k'_j / 8 (k' biasless)
    u = Wk[:, cols] @ bq[cols]
    kbq = (pre_value_key[b] @ u) / 8.0
    keyb = logm + kbq.astype(np.float32)
    logm_st = np.ascontiguousarray(keyb.reshape(16, 128).T.astype(f))
    b8m = B8 + A8 * keyb
    b8_st = np.ascontiguousarray(b8m.reshape(16, 128).T.astype(f))

    # ---- host gate (exact, generic) ----
    msum = np.float32(mask_b.sum())
    km256 = (mask_b @ pre_value_key[b]) @ (Wk[:, cols] / 8.0) \
        + (bk[cols] / 8.0) * msum
    gain = overall_gain.reshape(H)
    bias = overall_bias.reshape(H)
    cnt = np.float32(value_key_counts[b])
    km2 = km256.reshape(HPC, DH)
    U = np.einsum("dhk,hk->dh", Wq[:, cols].reshape(DQ, HPC, DH), km2)
    C = (bq[cols].reshape(HPC, DH) * km2).sum(1)
    pooled = pre_query[b] @ U + C
    z = pooled * (gain[h0 : h0 + HPC] / cnt) + bias[h0 : h0 + HPC]
    w = 1.0 / (1.0 + np.exp(-z.astype(np.float64)))
    w = w.astype(np.float32)  # [SQ, HPC]

    wg = np.ascontiguousarray(
        w.T.reshape(HPC, 4, 128, 4).transpose(2, 0, 1, 3))
    pq_split = pre_query[b, :, cols].reshape(SQ, HPC, DH)
    bv_h = bv[cols].reshape(HPC, DH)
    pqm0 = pq_split * (1.0 - w)[:, :, None] + bv_h[None] * w[:, :, None]
    pqm0T = np.ascontiguousarray(pqm0.reshape(SQ, 256).T)

    return {
        "pqT": pqT.astype(f8),
        "pvkT": pvkT.astype(f8),
        "wq": np.clip(wq, -240, 240).astype(f8),
        "wk": np.clip(wk, -240, 240).astype(f8),
        "wv": np.clip(wv, -240, 240).astype(f8),
        "logm": logm_st,
        "b8": b8_st,
        "wg": wg.astype(f, copy=False),
        "pqm0": pqm0T.astype(bf),
    }


def kernel(trace=False, **inputs):
    from concourse.bass_utils import run_bass_kernel_spmd

    inputs = {k: np.asarray(v, np.float32) for k, v in inputs.items()}
    nc = _get_nc()
    in_maps = [_prep_core_inputs(c, **inputs) for c in range(NCORES)]
    res = run_bass_kernel_spmd(nc, in_maps, core_ids=list(range(NCORES)),
                               trace=trace)
    _CACHE["last_result"] = res

    pre_query = inputs["pre_query"]
    out = np.empty((B, SQ, DQ), np.float32)
    out[:, :, DO:] = pre_query[:, :, DO:]
    for c in range(NCORES):
        b = c // 4
        h0 = (c % 4) * HPC
        oT = res.results[c]["outT"]
        out[b, :, h0 * DH : h0 * DH + HPC * DH] = oT.T
    return out


# revision 11
# speedup vs baseline: 1.1184x; 1.0131x over previous
"""Trainium2 Bass kernel for nn_MultiHeadedSelfAttention_86388972192276 (v4).

Sharding: 8 cores = 2 batches x 4 head-groups (4 heads each). Fully data
parallel, no collectives.

v4 over v3 (~175us):
  - Partial-K startup: attention begins after only kproj of the first
    key half (pv cv0) + qproj(0); kproj of cv1 is woven into step 0
    (kt2/kt4). First scores ~13us instead of ~31us (startup was
    DMA-bandwidth-bound at ~180 GB/s effective).
  - Numerator pair rotation (p1..p7, p0 last): kills the hps-WAR stall
    at step starts and lets step 0 consume v tiles just-in-time.
  - qproj split into single-pr chunks, one per step (si1..si6, kt5):
    only one PSUM slot borrowed at a time.
  - Blend: both-heads-in-one bounces (ld/lz/md [.., 2, ..]), w/l
    multiply applied via gpsimd CCE accum-DMA (mult) in place on the
    evacuated hcp tile, out = hcp + btP (bf16 pqm0) on gpsimd, plain
    write. No outT prefill, no accum RMW on DRAM, no Ln/Exp tail (one
    ACT table load total).
  - hps evacuation: both hh on ACT, emitted after the next step's
    first exp (no head-of-line blocking of exps).
  - exp split 8 ACT / 8 DVE alternating.

Bias handling (exact, host-folded):
  bk: shifts every score of a query equally -> softmax-invariant, drop.
  bq: adds bq.k'_j/8 per KEY -> folded into logm_eff / b8_eff.
  bv: h_full/l = h/l + bv -> folded into pqm0 as + bv*w.

Scale folding: wq_eff=16Wq, wk_eff=16Wk -> scores PSUM = 2048*s_true;
  wv_eff=32Wv and ones-col=32 -> v_dev = 32v, cancels in h/l.
"""

import sys
import numpy as np

sys.path.insert(0, "/opt/trn_rl_repo")

B, SQ, SV = 2, 2048, 2048
DV, DQ, DK, DO, H = 1024, 1280, 1024, 1024, 16
DH = 64
NCORES = 8
HPC = 4
NEG_MASK = -30000.0

SQW = 16.0
SKW = 16.0
SVW = 32.0
SSC = SQW * SKW * 8.0        # PSUM score = SSC * s_true
A8 = 8.0 / np.log(2.0)       # 11.5416...
B8 = 7.0 * 8.0 - 0.344       # e4m3 exp bias<<3, schraudolph-tuned

# engine per key-tile for exp: 'A' = ACT, 'D' = DVE (8/8 alternating)
ENG = ['A', 'D'] * 8

_CACHE = {}


def _build_nc():
    import concourse.bass as bass
    import concourse.tile as tile
    import concourse.mybir as mybir
    from concourse import bacc
    from contextlib import ExitStack

    fp32 = mybir.dt.float32
    bf16 = mybir.dt.bfloat16
    fp8 = mybir.dt.float8e4
    i8 = mybir.dt.int8
    AF = mybir.ActivationFunctionType
    ALU = mybir.AluOpType
    DR = mybir.MatmulPerfMode.DoubleRow

    nc = bacc.Bacc(None)

    pqT = nc.dram_tensor("pqT", [128, 10, SQ], fp8, kind="ExternalInput")
    pvkT = nc.dram_tensor("pvkT", [128, 8, SV], fp8, kind="ExternalInput")
    wq_d = nc.dram_tensor("wq", [128, 10, 256], fp8, kind="ExternalInput")
    wk_d = nc.dram_tensor("wk", [128, 8, 256], fp8, kind="ExternalInput")
    wv_d = nc.dram_tensor("wv", [128, 8, 256], fp8, kind="ExternalInput")
    logm_d = nc.dram_tensor("logm", [128, 16], fp32, kind="ExternalInput")
    b8_d = nc.dram_tensor("b8", [128, 16], fp32, kind="ExternalInput")
    wg_d = nc.dram_tensor("wg", [128, HPC, 4, 4], fp32, kind="ExternalInput")
    pqm0_d = nc.dram_tensor("pqm0", [HPC * DH, SQ], bf16, kind="ExternalInput")
    outT = nc.dram_tensor("outT", [HPC * DH, SQ], fp32, kind="ExternalOutput")

    with tile.TileContext(nc) as tc, ExitStack() as ctx:
        const = ctx.enter_context(tc.tile_pool(name="const", bufs=1))
        persist = ctx.enter_context(tc.tile_pool(name="persist", bufs=1))
        wpool = ctx.enter_context(tc.tile_pool(name="wpool", bufs=1))
        stream = ctx.enter_context(tc.tile_pool(name="stream", bufs=2))
        qstream = ctx.enter_context(tc.tile_pool(name="qstream", bufs=2))
        epool = ctx.enter_context(tc.tile_pool(name="epool", bufs=5))
        rows = ctx.enter_context(tc.tile_pool(name="rows", bufs=2))
        blend = ctx.enter_context(tc.tile_pool(name="blend", bufs=2))
        dscr = ctx.enter_context(tc.tile_pool(name="dscr", bufs=2,
                                              space="DRAM"))
        # PSUM (8 banks): scores 3x2 + numerator 2x1
        scps = ctx.enter_context(tc.tile_pool(name="scps", bufs=3,
                                              space="PSUM"))
        hps_p = ctx.enter_context(tc.tile_pool(name="hps", bufs=1,
                                               space="PSUM"))

        # ---- persistent activations (fp8) ----
        qT2 = [persist.tile([128, SQ], fp8, tag=f"qT2_{p}", name=f"qT2_{p}")
               for p in range(2)]
        kT2 = [persist.tile([128, SV], fp8, tag=f"kT2_{p}", name=f"kT2_{p}")
               for p in range(2)]
        # v_all[kk, svt, ch, 68]; col 64 = 32-ones (memset once);
        # cols 65-67 pad the ch-block stride to 272 B (DR LDW needs
        # k-tile step % 16 == 0)
        v_all = persist.tile([128, 16, HPC, 68], fp8, tag="v_all")
        nc.gpsimd.memset(v_all[:, :, :, 64], float(SVW))

        # ---- weights + inputs. Startup criticals first: sync queue
        # carries wk/wq/pq0, scalar queue carries pv (cv0 then cv1). ----
        wk_sb = wpool.tile([128, 8, 256], fp8)
        nc.sync.dma_start(wk_sb[:, 0:4, :], wk_d[:, 0:4, :])
        pv_cs = []
        for cv in range(2):
            pv_c = stream.tile([128, 8, 1024], fp8, tag="pv", name=f"pv{cv}")
            for j in range(2):
                nc.scalar.dma_start(
                    pv_c[:, :, bass.ds(j * 512, 512)],
                    pvkT[:, :, bass.ds(cv * 1024 + j * 512, 512)])
            pv_cs.append(pv_c)
        nc.sync.dma_start(wk_sb[:, 4:8, :], wk_d[:, 4:8, :])
        logm_sb = const.tile([128, 16], fp32)
        nc.sync.dma_start(logm_sb[:], logm_d[:])
        b8_sb = const.tile([128, 16], fp32)
        nc.sync.dma_start(b8_sb[:], b8_d[:])
        wq_sb = wpool.tile([128, 10, 256], fp8)
        nc.sync.dma_start(wq_sb[:], wq_d[:])

        # ---- projections (borrow "sc" PSUM slots) ----
        def kproj_chunk(cv, j):
            for pr in range(2):
                ps = scps.tile([128, 2, 512], fp32, tag="sc")
                for t in range(4):
                    nc.tensor.matmul(
                        ps[:, 0, :],
                        wk_sb[:, 2 * t : 2 * t + 2,
                              pr * 128 : pr * 128 + 128],
                        pv_cs[cv][:, 2 * t : 2 * t + 2,
                                  bass.ds(j * 512, 512)],
                        start=(t == 0), stop=(t == 3),
                        perf_mode=DR,
                    )
                nc.scalar.copy(
                    kT2[pr][:, bass.ds(cv * 1024 + j * 512, 512)],
                    ps[:, 0, :])

        pq_cs = {}

        def load_pq_chunk(c):
            pq_c = qstream.tile([128, 10, 512], fp8, tag="pq", name=f"pq{c}")
            nc.sync.dma_start(pq_c[:], pqT[:, :, bass.ds(c * 512, 512)])
            pq_cs[c] = pq_c

        def qproj_half_mm(c, pr):
            ps = scps.tile([128, 2, 512], fp32, tag="sc")
            for t in range(5):
                nc.tensor.matmul(
                    ps[:, 0, :],
                    wq_sb[:, 2 * t : 2 * t + 2,
                          pr * 128 : pr * 128 + 128],
                    pq_cs[c][:, 2 * t : 2 * t + 2, :],
                    start=(t == 0), stop=(t == 4),
                    perf_mode=DR,
                )
            return ps

        def qproj_half_copy(ps, c, pr):
            nc.scalar.copy(qT2[pr][:, bass.ds(c * 512, 512)],
                           ps[:, 0, :])

        def qproj_half(c, pr):
            qproj_half_copy(qproj_half_mm(c, pr), c, pr)

        def vproj_tile(svt):
            cv, sv = divmod(svt, 8)
            ps = scps.tile([128, 2, 512], fp32, tag="sc")
            for t in range(4):
                nc.tensor.matmul(
                    ps[:, 0, 0:256],
                    pv_cs[cv][:, 2 * t : 2 * t + 2,
                              bass.ds(sv * 128, 128)],
                    wv_sb[:, 2 * t : 2 * t + 2, :],
                    start=(t == 0), stop=(t == 3),
                    perf_mode=DR,
                )
            eng = nc.scalar if svt & 1 == 0 else nc.vector
            if svt & 1 == 0:
                eng.copy(
                    v_all[:, svt, :, 0:64],
                    ps[:, 0, 0:256].rearrange("p (c f) -> p c f", c=4))
            else:
                eng.tensor_copy(
                    v_all[:, svt, :, 0:64],
                    ps[:, 0, 0:256].rearrange("p (c f) -> p c f", c=4))

        # prologue: only the cv0 half of K, q chunk 0 halves.
        # wv/wg load after pq0 on the sync queue (needed later).
        kproj_chunk(0, 0)
        load_pq_chunk(0)
        wv_sb = wpool.tile([128, 8, 256], fp8)
        nc.sync.dma_start(wv_sb[:], wv_d[:])
        wg_sb = const.tile([128, HPC, 4, 4], fp32)
        nc.sync.dma_start(wg_sb[:], wg_d[:])
        kproj_chunk(0, 1)
        qproj_half(0, 0)
        qproj_half(0, 1)
        for svt in range(8):
            vproj_tile(svt)

        # ---- attention: global pipelined loop over units g=(si,kt) ----
        combos = [(pr, c) for c in range(4) for pr in range(2)]
        LEAD = 2
        NG = 8 * 16

        ps_store = {}

        def emit_scores(g):
            si, kt = divmod(g, 16)
            pr, c = combos[si]
            ps = scps.tile([128, 2, 512], fp32, tag="sc")
            for hh in range(2):
                ro = 64 * hh
                nc.tensor.matmul(
                    ps[:, hh, :],
                    kT2[pr][ro : ro + 64, bass.ds(kt * 128, 128)],
                    qT2[pr][ro : ro + 64, bass.ds(c * 512, 512)],
                    start=True, stop=True,
                )
            ps_store[g] = ps

        def emit_exp(g, e_t):
            si, kt = divmod(g, 16)
            ps = ps_store.pop(g)
            tpl = kt & 1
            if ENG[kt] == 'A':
                nc.scalar.activation(
                    e_t[:, tpl, :, :], ps[:], AF.Exp,
                    bias=logm_sb[:, kt : kt + 1], scale=float(1.0 / SSC))
            else:
                nc.vector.tensor_scalar(
                    e_t[:, tpl, :, :].bitcast(i8), ps[:],
                    float(A8 / SSC), b8_sb[:, kt : kt + 1],
                    ALU.mult, ALU.add)

        # numerator pair rotation: p1..p7 first, p0 last (kt15) so the
        # hps WAR on step entry is off the critical path.
        def emit_numer(e_pair, hps2, pr, p, first, last):
            for hh in range(2):
                nc.tensor.matmul(
                    hps2[hh][:],
                    v_all[:, 2 * p : 2 * p + 2, 2 * pr + hh, 0:65],
                    e_pair[:, :, hh, :],
                    start=first, stop=last,
                    perf_mode=DR,
                )

        # ---- blend stages for the previous step's (pr, c) ----
        def blend_s0(st):
            # evacuate hps (both hh on ACT) + combined l-row bounce
            hp = st["hps"]
            st["hcpP"] = blend.tile([65, 2, 512], fp32, tag="hcpP",
                                    name="hcpP")
            nc.scalar.copy(st["hcpP"][:, 0, :], hp[0][:])
            nc.scalar.copy(st["hcpP"][:, 1, :], hp[1][:])
            ld = dscr.tile([1, 2, 512], fp32, tag="ld", name="ld")
            nc.scalar.dma_start(ld[:], st["hcpP"][64:65, :, :])
            st["ld"] = ld

        def blend_s0b(st):
            lz = rows.tile([128, 2, 4], fp32, tag="lz", name="lz")
            nc.scalar.dma_start(
                lz[:],
                st["ld"].rearrange("o hh (p f) -> p o hh f", f=4))
            st["lz"] = lz

        def blend_s1(st):
            # 1/l then *w (DVE, tiny), bounce to DRAM row form
            pr, c = st["prc"]
            rl = rows.tile([128, 2, 4], fp32, tag="rl", name="rl")
            nc.vector.reciprocal(rl[:], st["lz"][:])
            m8 = rows.tile([128, 2, 4], fp32, tag="m8", name="m8")
            nc.vector.tensor_tensor(
                m8[:], wg_sb[:, 2 * pr : 2 * pr + 2, c, :], rl[:], ALU.mult)
            md = dscr.tile([1, 2, 512], fp32, tag="md", name="md")
            nc.scalar.dma_start(
                md.rearrange("o hh (p f) -> p o hh f", f=4), m8[:])
            st["md"] = md

        def blend_s2(st):
            # broadcast w/l into a [64, 2, 512] SBUF tile
            m1b = rows.tile([64, 2, 512], fp32, tag="m1b", name="m1b")
            nc.scalar.dma_start(
                m1b[:], st["md"][0:1, :, :].to_broadcast((64, 2, 512)))
            st["m1b"] = m1b

        def blend_s3(st, final=False):
            # out = h*(w/l) + pqm0 ; plain write
            pr, c = st["prc"]
            eng = nc.vector if final else nc.gpsimd
            aP = blend.tile([64, 2, 512], fp32, tag="aP", name="aP")
            eng.tensor_tensor(aP[:], st["hcpP"][0:64, :, :], st["m1b"][:],
                              ALU.mult)
            oP = blend.tile([64, 2, 512], fp32, tag="oP", name="oP")
            eng.tensor_tensor(oP[:], aP[:], st["btP"][:], ALU.add)
            dst = outT[bass.ds(2 * pr * 64, 128), bass.ds(c * 512, 512)]
            nc.sync.dma_start(
                dst.rearrange("(hh d) q -> d hh q", hh=2), oP[:])

        # scores lookahead prologue
        for g in range(LEAD):
            emit_scores(g)

        pending = None   # blend state of the previous step
        cur = None
        hps2 = None
        e_t = None
        e_hold = None

        for g in range(NG):
            si, kt = divmod(g, 16)
            pr, c = combos[si]
            if kt == 0:
                hps2 = [hps_p.tile([65, 512], fp32, tag=f"hT{hh}",
                                   name=f"hT{hh}")
                        for hh in range(2)]
                cur = {"prc": (pr, c), "hps": hps2}
            if g + LEAD < NG:
                emit_scores(g + LEAD)
            if kt & 1 == 0:
                e_t = epool.tile([128, 2, 2, 512], fp8, tag="e",
                                 name=f"e{si}_{kt // 2}")
                if kt == 0:
                    e_hold = e_t
            emit_exp(g, e_t)
            if si == 0 and kt >= 8:
                vproj_tile(kt)
            if kt & 1 == 1 and kt >= 3:
                p = kt // 2
                emit_numer(e_t, hps2, pr, p, first=(p == 1), last=False)
                if kt == 15:
                    emit_numer(e_hold, hps2, pr, 0, first=False, last=True)
            # previous step's blend stages, spread through this step
            if pending is not None:
                if kt == 0:
                    blend_s0(pending)
                elif kt == 1:
                    blend_s0b(pending)
                elif kt == 7:
                    blend_s1(pending)
                elif kt == 8:
                    blend_s2(pending)
                elif kt == 10:
                    blend_s3(pending)
            # per-step btP prefetch (used by the NEXT step's blend_s3)
            if kt == 8:
                btP = blend.tile([64, 2, 512], fp32, tag="btP", name="btP")
                src_bt = pqm0_d[bass.ds(2 * pr * 64, 128),
                                bass.ds(c * 512, 512)]
                nc.gpsimd.dma_start(
                    btP[:], src_bt.rearrange("(hh d) q -> d hh q", hh=2))
                cur["btP"] = btP
            # deferred cv1 K-projection + staging
            if si == 0 and kt == 2:
                kproj_chunk(1, 0)
                load_pq_chunk(1)
            if si == 0 and kt == 4:
                kproj_chunk(1, 1)
            if 1 <= si <= 6 and kt == 5:
                qc, qpr = divmod(si + 1, 2)
                qproj_ps = qproj_half_mm(qc, qpr)
            if 1 <= si <= 6 and kt == 8:
                qc, qpr = divmod(si + 1, 2)
                qproj_half_copy(qproj_ps, qc, qpr)
            if si in (1, 3) and kt == 10:
                load_pq_chunk(si // 2 + 2)
            if kt == 15:
                pending = cur

        # final step's blend (tail)
        blend_s0(pending)
        blend_s0b(pending)
        blend_s1(pending)
        blend_s2(pending)
        blend_s3(pending, final=True)

    nc.finalize()
    return nc


def _get_nc():
    if "nc" not in _CACHE:
        _CACHE["nc"] = _build_nc()
    return _CACHE["nc"]


def _prep_core_inputs(c, pre_value_key, pre_query, value_key_masks,
                      value_key_counts, Wq, bq, Wk, bk, Wv, bv,
                      overall_gain, overall_bias):
    import ml_dtypes
    f8 = ml_dtypes.float8_e4m3
    bf = ml_dtypes.bfloat16
    f = np.float32

    b = c // 4
    h0 = (c % 4) * HPC
    cols = slice(h0 * DH, h0 * DH + HPC * DH)

    pqT = np.ascontiguousarray(
        pre_query[b].T.reshape(10, 128, SQ).transpose(1, 0, 2))
    pvkT = np.ascontiguousarray(
        pre_value_key[b].T.reshape(8, 128, SV).transpose(1, 0, 2))

    wq = np.ascontiguousarray(
        (Wq[:, cols] * SQW).reshape(10, 128, 256).transpose(1, 0, 2))
    wk = np.ascontiguousarray(
        (Wk[:, cols] * SKW).reshape(8, 128, 256).transpose(1, 0, 2))
    wv = np.ascontiguousarray(
        (Wv[:, cols] * SVW).reshape(8, 128, 256).transpose(1, 0, 2))

    mask_b = value_key_masks[b]
    logm = np.where(mask_b == 0, np.float32(NEG_MASK), np.float32(0.0))
    # fold bq: per-key addend bq . k'_j / 8 (k' biasless)
    u = Wk[:, cols] @ bq[cols]
    kbq = (pre_value_key[b] @ u) / 8.0
    keyb = logm + kbq.astype(np.float32)
    logm_st = np.ascontiguousarray(keyb.reshape(16, 128).T.astype(f))
    b8m = B8 + A8 * keyb
    b8_st = np.ascontiguousarray(b8m.reshape(16, 128).T.astype(f))

    # ---- host gate (exact, generic) ----
    msum = np.float32(mask_b.sum())
    km256 = (mask_b @ pre_value_key[b]) @ (Wk[:, cols] / 8.0) \
        + (bk[cols] / 8.0) * msum
    gain = overall_gain.reshape(H)
    bias = overall_bias.reshape(H)
    cnt = np.float32(value_key_counts[b])
    km2 = km256.reshape(HPC, DH)
    U = np.einsum("dhk,hk->dh", Wq[:, cols].reshape(DQ, HPC, DH), km2)
    C = (bq[cols].reshape(HPC, DH) * km2).sum(1)
    pooled = pre_query[b] @ U + C
    z = pooled * (gain[h0 : h0 + HPC] / cnt) + bias[h0 : h0 + HPC]
    w = 1.0 / (1.0 + np.exp(-z.astype(np.float64)))
    w = w.astype(np.float32)  # [SQ, HPC]

    wg = np.ascontiguousarray(
        w.T.reshape(HPC, 4, 128, 4).transpose(2, 0, 1, 3))
    pq_split = pre_query[b, :, cols].reshape(SQ, HPC, DH)
    bv_h = bv[cols].reshape(HPC, DH)
    pqm0 = pq_split * (1.0 - w)[:, :, None] + bv_h[None] * w[:, :, None]
    pqm0T = np.ascontiguousarray(pqm0.reshape(SQ, 256).T)

    return {
        "pqT": pqT.astype(f8),
        "pvkT": pvkT.astype(f8),
        "wq": np.clip(wq, -240, 240).astype(f8),
        "wk": np.clip(wk, -240, 240).astype(f8),
        "wv": np.clip(wv, -240, 240).astype(f8),
        "logm": logm_st,
        "b8": b8_st,
        "wg": wg.astype(f, copy=False),
        "pqm0": pqm0T.astype(bf),
    }


def kernel(trace=False, **inputs):
    from concourse.bass_utils import run_bass_kernel_spmd

    inputs = {k: np.asarray(v, np.float32) for k, v in inputs.items()}
    nc = _get_nc()
    in_maps = [_prep_core_inputs(c, **inputs) for c in range(NCORES)]
    res = run_bass_kernel_spmd(nc, in_maps, core_ids=list(range(NCORES)),
                               trace=trace)
    _CACHE["last_result"] = res

    pre_query = inputs["pre_query"]
    out = np.empty((B, SQ, DQ), np.float32)
    out[:, :, DO:] = pre_query[:, :, DO:]
    for c in range(NCORES):
        b = c // 4
        h0 = (c % 4) * HPC
        oT = res.results[c]["outT"]
        out[b, :, h0 * DH : h0 * DH + HPC * DH] = oT.T
    return out
